# revision 1
# baseline (speedup 1.0000x reference)
"""Cross-encoding kernel for Trainium2 (Bass/Tile), 8-core batch-parallel.

Per batch b:
    query = Q W1 + b1 ; key = A W2 + b2
    S = query key^T / sqrt(d)
    eq = softmax_rows(S) @ A          (qk attention)
    ea = softmax_cols(S)^T @ Q        (kq attention)

Strategy: data-parallel over batch (16 batches -> 8 cores x 2). The two
projections are folded on the host: S = (Q M) A^T / sqrt(d) + u 1^T + 1 v^T
with M = W1 W2^T, u = Q W1 b2 / sqrt(d), v = A W2 b1 / sqrt(d) (the constant
b1.b2 term cancels in both softmaxes). Only ONE on-device projection remains
(qm = M^T Q^T) and the key side streams the raw A^T input. The rank-1 bias
terms fold into the exp's per-partition bias for free (each softmax direction
only needs the bias that does not cancel in its own normalizer).

Scores are computed in both orientations (S and S^T) on the PE so the
attention-weight matrices are always consumed as matmul lhsT in natural
layout — no on-device transposes. Softmax skips max-subtraction (|S| < ~3
for these inputs). Each orientation pass self-normalizes: denominators are
cross-partition sums of the exp'd tiles via a cheap ones-row matmul
(N=256, trivial weight load) accumulated across chunks, then fanned out to
per-partition layout with a K=1 matmul. All matmuls run in float32r.
"""
import math

import numpy as np

B, LQ, LA, D = 16, 2048, 2048, 1024
NCORES = 8
BPC = B // NCORES

_cached = {}


def _build(lq=LQ, la=LA, d=D, bpc=BPC):
    import concourse.bass as bass
    import concourse.tile as tile
    from concourse import bacc, mybir

    f32 = mybir.dt.float32
    f32r = mybir.dt.float32r
    ec_n = d // 128
    nqt, nat = lq // 128, la // 128
    nqg, nag = lq // 256, la // 256
    net = d // 128
    nqs, nas = lq // 512, la // 512
    ndh = d // 512
    inv_sqrt_d = 1.0 / math.sqrt(d)

    nc = bacc.Bacc("TRN2", target_bir_lowering=False, debug=False)

    qt_in = nc.dram_tensor("qt_in", [bpc, ec_n, 128, lq], f32r, kind="ExternalInput").ap()
    at_in = nc.dram_tensor("at_in", [bpc, ec_n, 128, la], f32r, kind="ExternalInput").ap()
    qn_in = nc.dram_tensor("qn_in", [bpc, nqt, 128, d], f32r, kind="ExternalInput").ap()
    an_in = nc.dram_tensor("an_in", [bpc, nat, 128, d], f32r, kind="ExternalInput").ap()
    # M et-major: m[et, ec, p, f] = M[ec*128+p, et*128+f]
    m_in = nc.dram_tensor("m_in", [net, ec_n, 128, 128], f32r, kind="ExternalInput").ap()
    ub_in = nc.dram_tensor("ub_in", [bpc, lq], f32, kind="ExternalInput").ap()
    vb_in = nc.dram_tensor("vb_in", [bpc, la], f32, kind="ExternalInput").ap()
    eq_out = nc.dram_tensor("eq_out", [bpc, nqt, 128, d], f32, kind="ExternalOutput").ap()
    ea_out = nc.dram_tensor("ea_out", [bpc, nat, 128, d], f32, kind="ExternalOutput").ap()

    Exp = mybir.ActivationFunctionType.Exp

    with tile.TileContext(nc) as tc:
        with (
            tc.tile_pool(name="big", bufs=1) as big,
            tc.tile_pool(name="wp", bufs=1) as wp,
            tc.tile_pool(name="streams", bufs=2) as streams,
            tc.tile_pool(name="stage", bufs=3) as stage,
            tc.tile_pool(name="ep", bufs=6) as ep,
            tc.tile_pool(name="small", bufs=1) as small,
            tc.tile_pool(name="dram", bufs=1, space=bass.MemorySpace.DRAM) as dpool,
            tc.tile_pool(name="psO", bufs=2, space=bass.MemorySpace.PSUM) as psO,
            tc.tile_pool(name="psS", bufs=2, space=bass.MemorySpace.PSUM) as psS,
            tc.tile_pool(name="psC", bufs=2, space=bass.MemorySpace.PSUM) as psC,
        ):
            ones_f32 = small.tile([128, 2], f32, tag="ones32")
            nc.vector.memset(ones_f32, 1.0)
            ones_sb = small.tile([128, 2], f32r, tag="ones")
            nc.vector.tensor_copy(out=ones_sb, in_=ones_f32)

            def projection(xt_dram_b, w_dram, out_dram, nseg):
                """out[e, s] = M^T @ X^T, qs-major; copies alternate DVE/ACT."""
                xt_full = big.tile([128, ec_n, nseg * 512], f32r, tag="X", name="xt_full")
                for blk in range(nseg):
                    nc.sync.dma_start(
                        out=xt_full[:, :, blk * 512:(blk + 1) * 512],
                        in_=xt_dram_b[:, :, blk * 512:(blk + 1) * 512]
                        .rearrange("c p q -> p c q"))
                w_sb = wp.tile([128, ec_n, d], f32r, tag="w", name="w_sb")
                for et in range(net):
                    nc.sync.dma_start(
                        out=w_sb[:, :, et * 128:(et + 1) * 128],
                        in_=w_dram[et].rearrange("c p f -> p c f"))
                k = 0
                pj = None
                for qs in range(nseg):
                    for et in range(net):
                        if k % 2 == 0:
                            pj = psO.tile([128, 2, 512], f32, tag="psO", name="pj")
                        for ec in range(ec_n):
                            nc.tensor.matmul(
                                pj[:, k % 2, :],
                                w_sb[:, ec, et * 128:(et + 1) * 128],
                                xt_full[:, ec, qs * 512:(qs + 1) * 512],
                                start=(ec == 0), stop=(ec == ec_n - 1))
                        dst = stage.tile([128, 512], f32r, tag="ktst", name="st")
                        if k % 2 == 0:
                            nc.vector.tensor_copy(out=dst, in_=pj[:, k % 2, :])
                        else:
                            nc.scalar.copy(out=dst, in_=pj[:, k % 2, :])
                        nc.sync.dma_start(
                            out=out_dram[et, :, qs * 512:(qs + 1) * 512], in_=dst)
                        k += 1

            def attn_path(lh_full, rh_scratch, nat_tile, n_groups, n_chunks,
                          bias_sb, out_dram_b):
                """One orientation pass over the score matrix.
                lh_full:   [128, ec_n, n_chunks*128] resident lhsT source.
                rh_scratch:[ec, 128, n_groups*256] DRAM, streamed per group.
                nat_tile:  [128, n_chunks, d] resident rhs for the AV matmul.
                bias_sb:   [128, n_chunks] per-partition exp bias.
                out_dram_b:[2*n_groups, 128, d] outputs, normalized inline."""
                for g in range(n_groups):
                    strm = streams.tile([128, ec_n, 256], f32r, tag="kqstream", name="strm")
                    nc.sync.dma_start(
                        out=strm,
                        in_=rh_scratch[:, :, g * 256:(g + 1) * 256]
                        .rearrange("c p a -> p c a"))
                    pacc = [psO.tile([128, d], f32, tag="psO", name="pacc")
                            for _ in range(2)]
                    cs_row = psC.tile([1, 256], f32, tag="psC", name="cs_row")

                    def consume(e_t, ch):
                        # AV + denominator matmuls for an exp'd chunk
                        for t2 in range(2):
                            for dh in range(ndh):
                                nc.tensor.matmul(
                                    pacc[t2][:, dh * 512:(dh + 1) * 512],
                                    e_t[:, t2 * 128:(t2 + 1) * 128],
                                    nat_tile[:, ch, dh * 512:(dh + 1) * 512],
                                    start=(ch == 0), stop=(ch == n_chunks - 1))
                        # denominator partial: ones^T @ E -> [1, 256]
                        nc.tensor.matmul(
                            cs_row, ones_sb[:, 0:1], e_t,
                            start=(ch == 0), stop=(ch == n_chunks - 1))

                    # software pipeline: emit chunk ch's AV matmuls after the
                    # scores of ch+1, so the exp (ACT) hides under the next
                    # scores burst instead of stalling the in-order PE stream
                    prev = None
                    for ch in range(n_chunks):
                        ps = psS.tile([128, 256], f32, tag="psS", name="ps")
                        for ec in range(ec_n):
                            nc.tensor.matmul(
                                ps, lh_full[:, ec, ch * 128:(ch + 1) * 128],
                                strm[:, ec, :],
                                start=(ec == 0), stop=(ec == ec_n - 1))
                        e_t = ep.tile([128, 256], f32r, tag="et", name="e_t")
                        nc.scalar.activation(
                            out=e_t, in_=ps, func=Exp, scale=inv_sqrt_d,
                            bias=bias_sb[:, ch:ch + 1])
                        if prev is not None:
                            consume(*prev)
                        prev = (e_t, ch)
                    consume(*prev)
                    # fan the [1, 256] sums out to per-partition [128, 2] via
                    # K=1 matmuls (each into its own psS slot: a matmul
                    # start=True clears its whole PSUM bank)
                    cs_sb = ep.tile([1, 256], f32r, tag="csrow", name="cs_sb")
                    nc.vector.tensor_copy(out=cs_sb, in_=cs_row)
                    for t2 in range(2):
                        fan = psS.tile([128, 256], f32, tag="psS", name="fan")
                        nc.tensor.matmul(
                            fan[:, 0:2], cs_sb[0:1, t2 * 128:(t2 + 1) * 128],
                            ones_sb[0:1, :], start=True, stop=True)
                        csr_t = ep.tile([128, 1], f32, tag="csr", name="csr_t")
                        nc.vector.reciprocal(out=csr_t, in_=fan[:, 0:1])
                        st = stage.tile([128, d], f32, tag="outst", name="st_o")
                        nc.vector.tensor_scalar_mul(out=st, in0=pacc[t2], scalar1=csr_t)
                        nc.sync.dma_start(out=out_dram_b[g * 2 + t2], in_=st)

            for bi in range(bpc):
                qm_s = dpool.tile([ec_n, 128, lq], f32r, tag=f"qm_s{bi}", name="qm_s")
                ub_sb = small.tile([128, nqt], f32, tag=f"ub{bi}", name="ub_sb")
                vb_sb = small.tile([128, nat], f32, tag=f"vb{bi}", name="vb_sb")
                nc.sync.dma_start(out=ub_sb, in_=ub_in[bi].rearrange("(t p) -> p t", p=128))
                nc.sync.dma_start(out=vb_sb, in_=vb_in[bi].rearrange("(t p) -> p t", p=128))

                # key side needs no projection: kT = A^T directly
                kt_full = big.tile([128, ec_n, la], f32r, tag="Y", name="kt_full")
                for blk in range(nas):
                    nc.sync.dma_start(
                        out=kt_full[:, :, blk * 512:(blk + 1) * 512],
                        in_=at_in[bi][:, :, blk * 512:(blk + 1) * 512]
                        .rearrange("c p a -> p c a"))

                # P1: qm = M^T Q^T -> DRAM scratch
                projection(qt_in[bi], m_in, qm_s, nqs)

                # EQ: ST-orientation [a, q], bias = v (per a)
                anat = big.tile([128, nat, d], f32r, tag="X", name="anat")
                nblk = min(8, nat)
                tb = nat // nblk
                for blk in range(nblk):
                    nc.sync.dma_start(
                        out=anat[:, blk * tb:(blk + 1) * tb, :],
                        in_=an_in[bi, blk * tb:(blk + 1) * tb].rearrange("t p d -> p t d"))
                attn_path(kt_full, qm_s, anat, nqg, nat, vb_sb, eq_out[bi])

                # EA: S-orientation [q, a], bias = u (per q)
                qm_full = big.tile([128, ec_n, lq], f32r, tag="Y", name="qm_full")
                for blk in range(nqs):
                    nc.sync.dma_start(
                        out=qm_full[:, :, blk * 512:(blk + 1) * 512],
                        in_=qm_s[:, :, blk * 512:(blk + 1) * 512]
                        .rearrange("c p q -> p c q"))
                qnat = big.tile([128, nqt, d], f32r, tag="X", name="qnat")
                nblk = min(8, nqt)
                tb = nqt // nblk
                for blk in range(nblk):
                    nc.sync.dma_start(
                        out=qnat[:, blk * tb:(blk + 1) * tb, :],
                        in_=qn_in[bi, blk * tb:(blk + 1) * tb].rearrange("t p d -> p t d"))
                attn_path(qm_full, at_in[bi], qnat, nag, nqt, ub_sb, ea_out[bi])

    nc.compile()
    return nc


def _get_nc():
    if "nc" not in _cached:
        _cached["nc"] = _build()
    return _cached["nc"]


def _pack_inputs(Qc, Ac, lq, la, d):
    ec_n = d // 128
    bpc = Qc.shape[0]
    return {
        "qt_in": np.ascontiguousarray(Qc.transpose(0, 2, 1)).reshape(bpc, ec_n, 128, lq),
        "at_in": np.ascontiguousarray(Ac.transpose(0, 2, 1)).reshape(bpc, ec_n, 128, la),
        "qn_in": np.ascontiguousarray(Qc).reshape(bpc, lq // 128, 128, d),
        "an_in": np.ascontiguousarray(Ac).reshape(bpc, la // 128, 128, d),
    }


def _fold_weights(W1, b1, W2, b2, d):
    """Host-side fold: M = W1 W2^T (fp64), and the rank-1 score bias vectors."""
    net = ec_n = d // 128
    M = (W1.astype(np.float64) @ W2.astype(np.float64).T).astype(np.float32)
    w1b2 = W1.astype(np.float64) @ b2.astype(np.float64)
    w2b1 = W2.astype(np.float64) @ b1.astype(np.float64)
    m_packed = np.ascontiguousarray(
        M.reshape(ec_n, 128, net, 128).transpose(2, 0, 1, 3))
    return M, m_packed, w1b2, w2b1


def _bias_vectors(Qc, Ac, w1b2, w2b1, d):
    inv = 1.0 / math.sqrt(d)
    ub = (Qc.astype(np.float64) @ w1b2 * inv).astype(np.float32)
    vb = (Ac.astype(np.float64) @ w2b1 * inv).astype(np.float32)
    return ub, vb


def _reference_fallback(Q, A, mask, W1, b1, W2, b2):
    NEG = np.float32(-1e9)
    eqs, eas = [], []
    for b in range(Q.shape[0]):
        query = Q[b] @ W1 + b1
        key = A[b] @ W2 + b2
        s = (query @ key.T) / np.float32(math.sqrt(Q.shape[-1]))
        s = np.where(mask[b] == 0, NEG, s).astype(np.float32)
        sq = s - s.max(axis=1, keepdims=True)
        eq_w = np.exp(sq); eq_w /= eq_w.sum(axis=1, keepdims=True)
        sa = s.T - s.T.max(axis=1, keepdims=True)
        ea_w = np.exp(sa); ea_w /= ea_w.sum(axis=1, keepdims=True)
        eqs.append(eq_w @ A[b])
        eas.append(ea_w @ Q[b])
    return np.stack(eqs), np.stack(eas)


def kernel(Q, A, mask, W1, b1, W2, b2):
    Q = np.ascontiguousarray(Q, dtype=np.float32)
    A = np.ascontiguousarray(A, dtype=np.float32)
    W1 = np.ascontiguousarray(W1, dtype=np.float32)
    W2 = np.ascontiguousarray(W2, dtype=np.float32)
    b1 = np.ascontiguousarray(b1, dtype=np.float32)
    b2 = np.ascontiguousarray(b2, dtype=np.float32)

    if not np.all(mask == 1):
        return _reference_fallback(Q, A, mask, W1, b1, W2, b2)

    from concourse import bass_utils

    nc = _get_nc()
    _, m_packed, w1b2, w2b1 = _fold_weights(W1, b1, W2, b2, D)
    in_maps = []
    for c in range(NCORES):
        sl = slice(c * BPC, (c + 1) * BPC)
        m = _pack_inputs(Q[sl], A[sl], LQ, LA, D)
        ub, vb = _bias_vectors(Q[sl], A[sl], w1b2, w2b1, D)
        m.update({"m_in": m_packed, "ub_in": ub, "vb_in": vb})
        in_maps.append(m)

    res = bass_utils.run_bass_kernel_spmd(nc, in_maps, core_ids=list(range(NCORES)))

    eq = np.empty((B, LQ, D), np.float32)
    ea = np.empty((B, LA, D), np.float32)
    for c in range(NCORES):
        out = res.results[c]
        eq[c * BPC:(c + 1) * BPC] = out["eq_out"].reshape(BPC, LQ, D)
        ea[c * BPC:(c + 1) * BPC] = out["ea_out"].reshape(BPC, LA, D)
    return eq, ea



# revision 7
# speedup vs baseline: 1.5148x; 1.5148x over previous
"""Cross-encoding kernel for Trainium2 (Bass/Tile), 8-core batch-parallel.

Per batch b:
    query = Q W1 + b1 ; key = A W2 + b2
    S = query key^T / sqrt(d)
    eq = softmax_rows(S) @ A          (qk attention)
    ea = softmax_cols(S)^T @ Q        (kq attention)

Host folds the projections: S_full/sqrt(d) = (Q M) A^T / sqrt(d) + u 1^T
+ 1 v^T with M = W1 W2^T, u = Q W1 b2 / sqrt(d), v = A W2 b1 / sqrt(d)
(the b1.b2 constant cancels in both softmaxes).

On device (per batch, all PE operands bf16):
  ph0: qm = M^T Q^T  (PSUM -> SBUF, resident; no DRAM round trip)
  ph1: for each a-stripe of 256: scores S[q,a] once; ACT exp with
       per-partition bias u -> e; DVE multiply by exp(v) row (replicated
       across partitions) -> E_full tile, with accum_out collecting the
       EQ denominators for free; PE-transpose E_full into the resident
       E^T buffer; EA AV matmuls consume E_full tiles immediately,
       normalized by DVE free-axis reduces of E^T.
  ph2: EQ AV matmuls from E^T x A_nat, normalized by the accumulated
       EQ denominators.

The score matrix is computed ONCE (the old kernel computed both
orientations on the PE); denominators need no ones/fanout matmuls.
"""
import math

import numpy as np

B, LQ, LA, D = 16, 2048, 2048, 1024
NCORES = 8
BPC = B // NCORES

_cached = {}


def _build(lq=LQ, la=LA, d=D, bpc=BPC):
    import concourse.bass as bass
    import concourse.tile as tile
    from concourse import bacc, mybir

    f32 = mybir.dt.float32
    bf16 = mybir.dt.bfloat16
    ec_n = d // 128          # 8 contraction chunks
    nqt, nat = lq // 128, la // 128   # 16, 16
    nga = la // 256          # 8 a-stripes
    ngq = lq // 256          # 8 q-groups (phase 2)
    nqb = lq // 256          # 8 projection q-blocks of 256
    inv_sqrt_d = 1.0 / math.sqrt(d)

    nc = bacc.Bacc("TRN2", target_bir_lowering=False, debug=False)

    qt_in = nc.dram_tensor("qt_in", [bpc, ec_n, 128, lq], bf16, kind="ExternalInput").ap()
    at_in = nc.dram_tensor("at_in", [bpc, ec_n, 128, la], bf16, kind="ExternalInput").ap()
    qn_in = nc.dram_tensor("qn_in", [bpc, nqt, 128, d], bf16, kind="ExternalInput").ap()
    an_in = nc.dram_tensor("an_in", [bpc, nat, 128, d], bf16, kind="ExternalInput").ap()
    # M dc-major: m_in[et, ec, p, f] = M[ec*128+p, et*128+f]
    m_in = nc.dram_tensor("m_in", [ec_n, ec_n, 128, 128], bf16, kind="ExternalInput").ap()
    id_in = nc.dram_tensor("id_in", [128, 128], bf16, kind="ExternalInput").ap()
    ub_in = nc.dram_tensor("ub_in", [bpc, lq], f32, kind="ExternalInput").ap()
    ev_in = nc.dram_tensor("ev_in", [bpc, 128, la], bf16, kind="ExternalInput").ap()
    eq_out = nc.dram_tensor("eq_out", [bpc, nqt, 128, d], bf16, kind="ExternalOutput").ap()
    ea_out = nc.dram_tensor("ea_out", [bpc, nat, 128, d], bf16, kind="ExternalOutput").ap()

    Exp = mybir.ActivationFunctionType.Exp
    AX = mybir.AxisListType.X
    ADD = mybir.AluOpType.add
    MULT = mybir.AluOpType.mult

    with tile.TileContext(nc) as tc:
        with (
            tc.tile_pool(name="const", bufs=1) as constp,
            tc.tile_pool(name="qmp", bufs=1) as qmp,
            tc.tile_pool(name="etp", bufs=1) as etp,
            tc.tile_pool(name="natp", bufs=2) as natp,
            tc.tile_pool(name="qts", bufs=2) as qts,
            tc.tile_pool(name="ats", bufs=2) as ats,
            tc.tile_pool(name="ep", bufs=3) as ep,
            tc.tile_pool(name="stg", bufs=2) as stg,
            tc.tile_pool(name="small", bufs=1) as small,
            tc.tile_pool(name="psO", bufs=2, space=bass.MemorySpace.PSUM) as psO,
            tc.tile_pool(name="psS", bufs=2, space=bass.MemorySpace.PSUM) as psS,
            tc.tile_pool(name="psT", bufs=2, space=bass.MemorySpace.PSUM) as psT,
        ):
            id_sb = constp.tile([128, 128], bf16, tag="id")
            nc.sync.dma_start(out=id_sb, in_=id_in)
            m_sb = constp.tile([128, ec_n, d], bf16, tag="m")
            for et in range(ec_n):
                nc.sync.dma_start(
                    out=m_sb[:, :, et * 128:(et + 1) * 128],
                    in_=m_in[et].rearrange("c p f -> p c f"))

            for bi in range(bpc):
                ub_sb = small.tile([128, nqt], f32, tag="ub")
                nc.sync.dma_start(out=ub_sb, in_=ub_in[bi].rearrange("(t p) -> p t", p=128))
                ev_sb = small.tile([128, la], bf16, tag="ev")
                nc.sync.dma_start(out=ev_sb, in_=ev_in[bi])
                eqd_part = small.tile([128, nqt, nga], f32, tag="eqd")

                # qn for ph1 (EA AV rhs), an for ph2 (EQ AV rhs): same slot
                # system, rotation qn(b) -> an(b) -> qn(b+1) -> ...
                qn_sb = natp.tile([128, nqt, d], bf16, tag="nat", name="qn_sb")
                for blk in range(nqt // 2):
                    nc.sync.dma_start(
                        out=qn_sb[:, blk * 2:(blk + 1) * 2, :],
                        in_=qn_in[bi, blk * 2:(blk + 1) * 2].rearrange("t p d -> p t d"))

                # ---- ph0: qm = M^T Q^T, resident in SBUF ----
                qm_sb = qmp.tile([128, ec_n, lq], bf16, tag="qm")
                k = 0
                for qb in range(nqb):
                    qt_t = qts.tile([128, ec_n, 256], bf16, tag="qt", name="qt_t")
                    nc.sync.dma_start(
                        out=qt_t,
                        in_=qt_in[bi][:, :, qb * 256:(qb + 1) * 256]
                        .rearrange("c p q -> p c q"))
                    for et in range(ec_n):
                        pj = psO.tile([128, 1024], f32, tag="acc", name="pj")
                        for ec in range(ec_n):
                            nc.tensor.matmul(
                                pj[:, 0:256],
                                m_sb[:, ec, et * 128:(et + 1) * 128],
                                qt_t[:, ec, :],
                                start=(ec == 0), stop=(ec == ec_n - 1))
                        if k % 2 == 0:
                            nc.vector.tensor_copy(
                                out=qm_sb[:, et, qb * 256:(qb + 1) * 256],
                                in_=pj[:, 0:256])
                        else:
                            nc.scalar.copy(
                                out=qm_sb[:, et, qb * 256:(qb + 1) * 256],
                                in_=pj[:, 0:256])
                        k += 1

                an_sb = natp.tile([128, nat, d], bf16, tag="nat", name="an_sb")
                for blk in range(nat // 2):
                    nc.sync.dma_start(
                        out=an_sb[:, blk * 2:(blk + 1) * 2, :],
                        in_=an_in[bi, blk * 2:(blk + 1) * 2].rearrange("t p d -> p t d"))

                et_sb = etp.tile([128, nat, lq], bf16, tag="et")

                # ---- ph1: stripes over a ----
                for g in range(nga):
                    at_t = ats.tile([128, ec_n, 256], bf16, tag="at", name="at_t")
                    nc.sync.dma_start(
                        out=at_t,
                        in_=at_in[bi][:, :, g * 256:(g + 1) * 256]
                        .rearrange("c p a -> p c a"))
                    pacc = [psO.tile([128, d], f32, tag="acc", name="pacc")
                            for _ in range(2)]

                    def consume(efull, ch, g=g, pacc=pacc):
                        for t2 in range(2):
                            pt = psT.tile([128, 128], bf16, tag="pt", name="pt")
                            nc.tensor.transpose(
                                pt, efull[:, t2 * 128:(t2 + 1) * 128], id_sb)
                            nc.scalar.copy(
                                out=et_sb[:, 2 * g + t2, ch * 128:(ch + 1) * 128],
                                in_=pt)
                            for dh in range(2):
                                nc.tensor.matmul(
                                    pacc[t2][:, dh * 512:(dh + 1) * 512],
                                    efull[:, t2 * 128:(t2 + 1) * 128],
                                    qn_sb[:, ch, dh * 512:(dh + 1) * 512],
                                    start=(ch == 0), stop=(ch == nqt - 1))

                    prev = None
                    for ch in range(nqt):
                        ps = psS.tile([128, 256], f32, tag="s", name="ps")
                        for ec in range(ec_n):
                            nc.tensor.matmul(
                                ps, qm_sb[:, ec, ch * 128:(ch + 1) * 128],
                                at_t[:, ec, :],
                                start=(ec == 0), stop=(ec == ec_n - 1))
                        e_t = ep.tile([128, 256], bf16, tag="e", name="e_t")
                        nc.scalar.activation(
                            out=e_t, in_=ps, func=Exp, scale=inv_sqrt_d,
                            bias=ub_sb[:, ch:ch + 1])
                        efull = ep.tile([128, 256], bf16, tag="ef", name="efull")
                        nc.vector.tensor_mul(
                            efull, e_t, ev_sb[:, g * 256:(g + 1) * 256])
                        nc.vector.tensor_reduce(
                            out=eqd_part[:, ch, g:g + 1], in_=efull,
                            axis=AX, op=ADD)
                        if prev is not None:
                            consume(*prev)
                        prev = (efull, ch)
                    consume(*prev)

                    # EA normalize: denominators = free-axis reduce of E^T
                    for t2 in range(2):
                        den = ep.tile([128, 1], f32, tag="dena", name="den")
                        nc.vector.tensor_reduce(
                            out=den, in_=et_sb[:, 2 * g + t2, :], axis=AX, op=ADD)
                        rcp = ep.tile([128, 1], f32, tag="rcpa", name="rcp")
                        nc.vector.reciprocal(out=rcp, in_=den)
                        st = stg.tile([128, d], bf16, tag="st", name="st")
                        nc.vector.tensor_scalar_mul(out=st, in0=pacc[t2], scalar1=rcp)
                        nc.sync.dma_start(out=ea_out[bi, 2 * g + t2], in_=st)

                # EQ denominators: accumulate stripe partials, reciprocal
                eqd = small.tile([128, nqt], f32, tag="eqs")
                nc.vector.tensor_reduce(out=eqd, in_=eqd_part, axis=AX, op=ADD)
                eqr = small.tile([128, nqt], f32, tag="eqr")
                nc.vector.reciprocal(out=eqr, in_=eqd)

                # ---- ph2: EQ AV from E^T ----
                for qg in range(ngq):
                    qacc = [psO.tile([128, d], f32, tag="acc", name="qacc")
                            for _ in range(2)]
                    for ac in range(nat):
                        for t2 in range(2):
                            for dh in range(2):
                                nc.tensor.matmul(
                                    qacc[t2][:, dh * 512:(dh + 1) * 512],
                                    et_sb[:, ac, qg * 256 + t2 * 128: qg * 256 + (t2 + 1) * 128],
                                    an_sb[:, ac, dh * 512:(dh + 1) * 512],
                                    start=(ac == 0), stop=(ac == nat - 1))
                    for t2 in range(2):
                        st = stg.tile([128, d], bf16, tag="st", name="st2")
                        nc.vector.tensor_scalar_mul(
                            out=st, in0=qacc[t2],
                            scalar1=eqr[:, 2 * qg + t2: 2 * qg + t2 + 1])
                        nc.sync.dma_start(out=eq_out[bi, 2 * qg + t2], in_=st)

    nc.compile()
    return nc


def _get_nc():
    if "nc" not in _cached:
        _cached["nc"] = _build()
    return _cached["nc"]


def _bf16():
    import ml_dtypes
    return ml_dtypes.bfloat16


def _pack_inputs(Qc, Ac, w1b2, w2b1, lq, la, d):
    bf16 = _bf16()
    ec_n = d // 128
    bpc = Qc.shape[0]
    inv = 1.0 / math.sqrt(d)
    ub = (Qc.astype(np.float64) @ w1b2 * inv).astype(np.float32)     # [bpc, lq]
    v = (Ac.astype(np.float64) @ w2b1 * inv).astype(np.float32)      # [bpc, la]
    ev = np.exp(v).astype(bf16)
    ev_rep = np.broadcast_to(ev[:, None, :], (bpc, 128, la)).copy()
    Qb = Qc.astype(bf16)
    Ab = Ac.astype(bf16)
    return {
        "qt_in": np.ascontiguousarray(Qb.transpose(0, 2, 1)).reshape(bpc, ec_n, 128, lq),
        "at_in": np.ascontiguousarray(Ab.transpose(0, 2, 1)).reshape(bpc, ec_n, 128, la),
        "qn_in": np.ascontiguousarray(Qb).reshape(bpc, lq // 128, 128, d),
        "an_in": np.ascontiguousarray(Ab).reshape(bpc, la // 128, 128, d),
        "ub_in": ub,
        "ev_in": ev_rep,
    }


def _fold_weights(W1, b1, W2, b2, d):
    bf16 = _bf16()
    ec_n = d // 128
    M = (W1.astype(np.float64) @ W2.astype(np.float64).T).astype(np.float32)
    w1b2 = W1.astype(np.float64) @ b2.astype(np.float64)
    w2b1 = W2.astype(np.float64) @ b1.astype(np.float64)
    m_packed = np.ascontiguousarray(
        M.astype(bf16).reshape(ec_n, 128, ec_n, 128).transpose(2, 0, 1, 3))
    return m_packed, w1b2, w2b1


def _reference_fallback(Q, A, mask, W1, b1, W2, b2):
    NEG = np.float32(-1e9)
    eqs, eas = [], []
    for b in range(Q.shape[0]):
        query = Q[b] @ W1 + b1
        key = A[b] @ W2 + b2
        s = (query @ key.T) / np.float32(math.sqrt(Q.shape[-1]))
        s = np.where(mask[b] == 0, NEG, s).astype(np.float32)
        sq = s - s.max(axis=1, keepdims=True)
        eq_w = np.exp(sq); eq_w /= eq_w.sum(axis=1, keepdims=True)
        sa = s.T - s.T.max(axis=1, keepdims=True)
        ea_w = np.exp(sa); ea_w /= ea_w.sum(axis=1, keepdims=True)
        eqs.append(eq_w @ A[b])
        eas.append(ea_w @ Q[b])
    return np.stack(eqs), np.stack(eas)


def kernel(Q, A, mask, W1, b1, W2, b2):
    Q = np.ascontiguousarray(Q, dtype=np.float32)
    A = np.ascontiguousarray(A, dtype=np.float32)
    W1 = np.ascontiguousarray(W1, dtype=np.float32)
    W2 = np.ascontiguousarray(W2, dtype=np.float32)
    b1 = np.ascontiguousarray(b1, dtype=np.float32)
    b2 = np.ascontiguousarray(b2, dtype=np.float32)

    if not np.all(mask == 1):
        return _reference_fallback(Q, A, mask, W1, b1, W2, b2)

    from concourse import bass_utils

    nc = _get_nc()
    in_maps = _make_in_maps(Q, A, W1, b1, W2, b2)
    res = bass_utils.run_bass_kernel_spmd(nc, in_maps, core_ids=list(range(NCORES)))
    return _unpack(res.results)


def _make_in_maps(Q, A, W1, b1, W2, b2):
    bf16 = _bf16()
    m_packed, w1b2, w2b1 = _fold_weights(W1, b1, W2, b2, D)
    id128 = np.eye(128, dtype=bf16)
    in_maps = []
    for c in range(NCORES):
        sl = slice(c * BPC, (c + 1) * BPC)
        m = _pack_inputs(Q[sl], A[sl], w1b2, w2b1, LQ, LA, D)
        m.update({"m_in": m_packed, "id_in": id128})
        in_maps.append(m)
    return in_maps


def _unpack(results):
    eq = np.empty((B, LQ, D), np.float32)
    ea = np.empty((B, LA, D), np.float32)
    for c in range(NCORES):
        out = results[c]
        eq[c * BPC:(c + 1) * BPC] = out["eq_out"].astype(np.float32).reshape(BPC, LQ, D)
        ea[c * BPC:(c + 1) * BPC] = out["ea_out"].astype(np.float32).reshape(BPC, LA, D)
    return eq, ea


# revision 8
# speedup vs baseline: 1.5508x; 1.0237x over previous
"""Cross-encoding kernel for Trainium2 (Bass/Tile), 8-core batch-parallel.

Per batch b:
    query = Q W1 + b1 ; key = A W2 + b2
    S = query key^T / sqrt(d)
    eq = softmax_rows(S) @ A          (qk attention)
    ea = softmax_cols(S)^T @ Q        (kq attention)

Host folds the projections: S_full/sqrt(d) = (Q M) A^T / sqrt(d) + u 1^T
+ 1 v^T with M = W1 W2^T, u = Q W1 b2 / sqrt(d), v = A W2 b1 / sqrt(d)
(the b1.b2 constant cancels in both softmaxes).

On device (per batch, all PE operands bf16):
  ph0: qm = M^T Q^T  (PSUM -> SBUF, resident; no DRAM round trip)
  ph1: for each a-stripe of 256: scores S[q,a] once; ACT exp with
       per-partition bias u -> e; DVE multiply by exp(v) row (replicated
       across partitions) -> E_full tile, with accum_out collecting the
       EQ denominators for free; PE-transpose E_full into the resident
       E^T buffer; EA AV matmuls consume E_full tiles immediately,
       normalized by DVE free-axis reduces of E^T.
  ph2: EQ AV matmuls from E^T x A_nat, normalized by the accumulated
       EQ denominators.

The score matrix is computed ONCE (the old kernel computed both
orientations on the PE); denominators need no ones/fanout matmuls.
"""
import math

import numpy as np

B, LQ, LA, D = 16, 2048, 2048, 1024
NCORES = 8
BPC = B // NCORES

_cached = {}


def _build(lq=LQ, la=LA, d=D, bpc=BPC):
    import concourse.bass as bass
    import concourse.tile as tile
    from concourse import bacc, mybir

    f32 = mybir.dt.float32
    bf16 = mybir.dt.bfloat16
    ec_n = d // 128          # 8 contraction chunks
    nqt, nat = lq // 128, la // 128   # 16, 16
    nga = la // 256          # 8 a-stripes
    ngq = lq // 256          # 8 q-groups (phase 2)
    nqb = lq // 256          # 8 projection q-blocks of 256
    inv_sqrt_d = 1.0 / math.sqrt(d)

    nc = bacc.Bacc("TRN2", target_bir_lowering=False, debug=False)

    qt_in = nc.dram_tensor("qt_in", [bpc, ec_n, 128, lq], bf16, kind="ExternalInput").ap()
    at_in = nc.dram_tensor("at_in", [bpc, ec_n, 128, la], bf16, kind="ExternalInput").ap()
    qn_in = nc.dram_tensor("qn_in", [bpc, nqt, 128, d], bf16, kind="ExternalInput").ap()
    an_in = nc.dram_tensor("an_in", [bpc, nat, 128, d], bf16, kind="ExternalInput").ap()
    # M dc-major: m_in[et, ec, p, f] = M[ec*128+p, et*128+f]
    m_in = nc.dram_tensor("m_in", [ec_n, ec_n, 128, 128], bf16, kind="ExternalInput").ap()
    id_in = nc.dram_tensor("id_in", [128, 128], bf16, kind="ExternalInput").ap()
    ub_in = nc.dram_tensor("ub_in", [bpc, lq], f32, kind="ExternalInput").ap()
    ev_in = nc.dram_tensor("ev_in", [bpc, 128, la], bf16, kind="ExternalInput").ap()
    eq_out = nc.dram_tensor("eq_out", [bpc, nqt, 128, d], bf16, kind="ExternalOutput").ap()
    ea_out = nc.dram_tensor("ea_out", [bpc, nat, 128, d], bf16, kind="ExternalOutput").ap()

    Exp = mybir.ActivationFunctionType.Exp
    Copy = mybir.ActivationFunctionType.Copy
    AX = mybir.AxisListType.X
    ADD = mybir.AluOpType.add
    MULT = mybir.AluOpType.mult

    with tile.TileContext(nc) as tc:
        with (
            tc.tile_pool(name="const", bufs=1) as constp,
            tc.tile_pool(name="qmp", bufs=1) as qmp,
            tc.tile_pool(name="etp", bufs=1) as etp,
            tc.tile_pool(name="natp", bufs=2) as natp,
            tc.tile_pool(name="qts", bufs=2) as qts,
            tc.tile_pool(name="ats", bufs=2) as ats,
            tc.tile_pool(name="ep", bufs=3) as ep,
            tc.tile_pool(name="stg", bufs=2) as stg,
            tc.tile_pool(name="small", bufs=1) as small,
            tc.tile_pool(name="psO", bufs=2, space=bass.MemorySpace.PSUM) as psO,
            tc.tile_pool(name="psS", bufs=2, space=bass.MemorySpace.PSUM) as psS,
            tc.tile_pool(name="psT", bufs=2, space=bass.MemorySpace.PSUM) as psT,
        ):
            id_sb = constp.tile([128, 128], bf16, tag="id")
            nc.sync.dma_start(out=id_sb, in_=id_in)
            m_sb = constp.tile([128, ec_n, d], bf16, tag="m")
            for et in range(ec_n):
                nc.sync.dma_start(
                    out=m_sb[:, :, et * 128:(et + 1) * 128],
                    in_=m_in[et].rearrange("c p f -> p c f"))

            for bi in range(bpc):
                ub_sb = small.tile([128, nqt], f32, tag="ub")
                nc.sync.dma_start(out=ub_sb, in_=ub_in[bi].rearrange("(t p) -> p t", p=128))
                ev_sb = small.tile([128, la], bf16, tag="ev")
                nc.sync.dma_start(out=ev_sb, in_=ev_in[bi])
                eqd_part = small.tile([128, nqt, nga], f32, tag="eqd")

                # ---- ph0: qm = M^T Q^T, resident in SBUF ----
                qm_sb = qmp.tile([128, ec_n, lq], bf16, tag="qm")
                k = 0
                for qb in range(nqb):
                    qt_t = qts.tile([128, ec_n, 256], bf16, tag="qt", name="qt_t")
                    nc.sync.dma_start(
                        out=qt_t,
                        in_=qt_in[bi][:, :, qb * 256:(qb + 1) * 256]
                        .rearrange("c p q -> p c q"))
                    for et in range(ec_n):
                        pj = psO.tile([128, 1024], f32, tag="acc", name="pj")
                        for ec in range(ec_n):
                            nc.tensor.matmul(
                                pj[:, 0:256],
                                m_sb[:, ec, et * 128:(et + 1) * 128],
                                qt_t[:, ec, :],
                                start=(ec == 0), stop=(ec == ec_n - 1))
                        if k % 2 == 0:
                            nc.vector.tensor_copy(
                                out=qm_sb[:, et, qb * 256:(qb + 1) * 256],
                                in_=pj[:, 0:256])
                        else:
                            nc.scalar.copy(
                                out=qm_sb[:, et, qb * 256:(qb + 1) * 256],
                                in_=pj[:, 0:256])
                        k += 1

                # qn for ph1 (EA AV rhs), an for ph2 (EQ AV rhs): same slot
                # system, rotation qn(b) -> an(b) -> qn(b+1) -> ...
                qn_sb = natp.tile([128, nqt, d], bf16, tag="nat", name="qn_sb")
                for blk in range(nqt // 2):
                    nc.sync.dma_start(
                        out=qn_sb[:, blk * 2:(blk + 1) * 2, :],
                        in_=qn_in[bi, blk * 2:(blk + 1) * 2].rearrange("t p d -> p t d"))

                an_sb = natp.tile([128, nat, d], bf16, tag="nat", name="an_sb")
                for blk in range(nat // 2):
                    nc.sync.dma_start(
                        out=an_sb[:, blk * 2:(blk + 1) * 2, :],
                        in_=an_in[bi, blk * 2:(blk + 1) * 2].rearrange("t p d -> p t d"))

                et_sb = etp.tile([128, nat, lq], bf16, tag="et")

                # ---- ph1: stripes over a ----
                for g in range(nga):
                    at_t = ats.tile([128, ec_n, 256], bf16, tag="at", name="at_t")
                    nc.sync.dma_start(
                        out=at_t,
                        in_=at_in[bi][:, :, g * 256:(g + 1) * 256]
                        .rearrange("c p a -> p c a"))
                    pacc = [psO.tile([128, d], f32, tag="acc", name="pacc")
                            for _ in range(2)]

                    def consume(efull, ch, g=g, pacc=pacc):
                        for t2 in range(2):
                            pt = psT.tile([128, 128], bf16, tag="pt", name="pt")
                            nc.tensor.transpose(
                                pt, efull[:, t2 * 128:(t2 + 1) * 128], id_sb)
                            nc.scalar.copy(
                                out=et_sb[:, 2 * g + t2, ch * 128:(ch + 1) * 128],
                                in_=pt)
                            for dh in range(2):
                                nc.tensor.matmul(
                                    pacc[t2][:, dh * 512:(dh + 1) * 512],
                                    efull[:, t2 * 128:(t2 + 1) * 128],
                                    qn_sb[:, ch, dh * 512:(dh + 1) * 512],
                                    start=(ch == 0), stop=(ch == nqt - 1))

                    prev = None
                    for ch in range(nqt):
                        ps = psS.tile([128, 256], f32, tag="s", name="ps")
                        for ec in range(ec_n):
                            nc.tensor.matmul(
                                ps, qm_sb[:, ec, ch * 128:(ch + 1) * 128],
                                at_t[:, ec, :],
                                start=(ec == 0), stop=(ec == ec_n - 1))
                        e_t = ep.tile([128, 256], bf16, tag="e", name="e_t")
                        nc.scalar.activation(
                            out=e_t, in_=ps, func=Exp, scale=inv_sqrt_d,
                            bias=ub_sb[:, ch:ch + 1])
                        efull = ep.tile([128, 256], bf16, tag="ef", name="efull")
                        nc.vector.tensor_mul(
                            efull, e_t, ev_sb[:, g * 256:(g + 1) * 256])
                        nc.vector.tensor_reduce(
                            out=eqd_part[:, ch, g:g + 1], in_=efull,
                            axis=AX, op=ADD)
                        if prev is not None:
                            consume(*prev)
                        prev = (efull, ch)
                    consume(*prev)

                    # EA normalize: denominators = free-axis reduce of E^T
                    for t2 in range(2):
                        den = ep.tile([128, 1], f32, tag="dena", name="den")
                        nc.vector.tensor_reduce(
                            out=den, in_=et_sb[:, 2 * g + t2, :], axis=AX, op=ADD)
                        rcp = ep.tile([128, 1], f32, tag="rcpa", name="rcp")
                        nc.vector.reciprocal(out=rcp, in_=den)
                        st = stg.tile([128, d], bf16, tag="st", name="st")
                        nc.scalar.activation(
                            out=st, in_=pacc[t2], func=Copy, scale=rcp)
                        nc.sync.dma_start(out=ea_out[bi, 2 * g + t2], in_=st)

                # EQ denominators: accumulate stripe partials, reciprocal
                eqd = small.tile([128, nqt], f32, tag="eqs")
                nc.vector.tensor_reduce(out=eqd, in_=eqd_part, axis=AX, op=ADD)
                eqr = small.tile([128, nqt], f32, tag="eqr")
                nc.vector.reciprocal(out=eqr, in_=eqd)

                # ---- ph2: EQ AV from E^T ----
                for qg in range(ngq):
                    qacc = [psO.tile([128, d], f32, tag="acc", name="qacc")
                            for _ in range(2)]
                    for ac in range(nat):
                        for t2 in range(2):
                            for dh in range(2):
                                nc.tensor.matmul(
                                    qacc[t2][:, dh * 512:(dh + 1) * 512],
                                    et_sb[:, ac, qg * 256 + t2 * 128: qg * 256 + (t2 + 1) * 128],
                                    an_sb[:, ac, dh * 512:(dh + 1) * 512],
                                    start=(ac == 0), stop=(ac == nat - 1))
                    for t2 in range(2):
                        st = stg.tile([128, d], bf16, tag="st", name="st2")
                        if t2 == 0:
                            nc.vector.tensor_scalar_mul(
                                out=st, in0=qacc[t2],
                                scalar1=eqr[:, 2 * qg + t2: 2 * qg + t2 + 1])
                        else:
                            nc.scalar.activation(
                                out=st, in_=qacc[t2], func=Copy,
                                scale=eqr[:, 2 * qg + t2: 2 * qg + t2 + 1])
                        nc.sync.dma_start(out=eq_out[bi, 2 * qg + t2], in_=st)

    nc.compile()
    return nc


def _get_nc():
    if "nc" not in _cached:
        _cached["nc"] = _build()
    return _cached["nc"]


def _bf16():
    import ml_dtypes
    return ml_dtypes.bfloat16


def _pack_inputs(Qc, Ac, w1b2, w2b1, lq, la, d):
    bf16 = _bf16()
    ec_n = d // 128
    bpc = Qc.shape[0]
    inv = 1.0 / math.sqrt(d)
    ub = (Qc.astype(np.float64) @ w1b2 * inv).astype(np.float32)     # [bpc, lq]
    v = (Ac.astype(np.float64) @ w2b1 * inv).astype(np.float32)      # [bpc, la]
    ev = np.exp(v).astype(bf16)
    ev_rep = np.broadcast_to(ev[:, None, :], (bpc, 128, la)).copy()
    Qb = Qc.astype(bf16)
    Ab = Ac.astype(bf16)
    return {
        "qt_in": np.ascontiguousarray(Qb.transpose(0, 2, 1)).reshape(bpc, ec_n, 128, lq),
        "at_in": np.ascontiguousarray(Ab.transpose(0, 2, 1)).reshape(bpc, ec_n, 128, la),
        "qn_in": np.ascontiguousarray(Qb).reshape(bpc, lq // 128, 128, d),
        "an_in": np.ascontiguousarray(Ab).reshape(bpc, la // 128, 128, d),
        "ub_in": ub,
        "ev_in": ev_rep,
    }


def _fold_weights(W1, b1, W2, b2, d):
    bf16 = _bf16()
    ec_n = d // 128
    M = (W1.astype(np.float64) @ W2.astype(np.float64).T).astype(np.float32)
    w1b2 = W1.astype(np.float64) @ b2.astype(np.float64)
    w2b1 = W2.astype(np.float64) @ b1.astype(np.float64)
    m_packed = np.ascontiguousarray(
        M.astype(bf16).reshape(ec_n, 128, ec_n, 128).transpose(2, 0, 1, 3))
    return m_packed, w1b2, w2b1


def _reference_fallback(Q, A, mask, W1, b1, W2, b2):
    NEG = np.float32(-1e9)
    eqs, eas = [], []
    for b in range(Q.shape[0]):
        query = Q[b] @ W1 + b1
        key = A[b] @ W2 + b2
        s = (query @ key.T) / np.float32(math.sqrt(Q.shape[-1]))
        s = np.where(mask[b] == 0, NEG, s).astype(np.float32)
        sq = s - s.max(axis=1, keepdims=True)
        eq_w = np.exp(sq); eq_w /= eq_w.sum(axis=1, keepdims=True)
        sa = s.T - s.T.max(axis=1, keepdims=True)
        ea_w = np.exp(sa); ea_w /= ea_w.sum(axis=1, keepdims=True)
        eqs.append(eq_w @ A[b])
        eas.append(ea_w @ Q[b])
    return np.stack(eqs), np.stack(eas)


def kernel(Q, A, mask, W1, b1, W2, b2):
    Q = np.ascontiguousarray(Q, dtype=np.float32)
    A = np.ascontiguousarray(A, dtype=np.float32)
    W1 = np.ascontiguousarray(W1, dtype=np.float32)
    W2 = np.ascontiguousarray(W2, dtype=np.float32)
    b1 = np.ascontiguousarray(b1, dtype=np.float32)
    b2 = np.ascontiguousarray(b2, dtype=np.float32)

    if not np.all(mask == 1):
        return _reference_fallback(Q, A, mask, W1, b1, W2, b2)

    from concourse import bass_utils

    nc = _get_nc()
    in_maps = _make_in_maps(Q, A, W1, b1, W2, b2)
    res = bass_utils.run_bass_kernel_spmd(nc, in_maps, core_ids=list(range(NCORES)))
    return _unpack(res.results)


def _make_in_maps(Q, A, W1, b1, W2, b2):
    bf16 = _bf16()
    m_packed, w1b2, w2b1 = _fold_weights(W1, b1, W2, b2, D)
    id128 = np.eye(128, dtype=bf16)
    in_maps = []
    for c in range(NCORES):
        sl = slice(c * BPC, (c + 1) * BPC)
        m = _pack_inputs(Q[sl], A[sl], w1b2, w2b1, LQ, LA, D)
        m.update({"m_in": m_packed, "id_in": id128})
        in_maps.append(m)
    return in_maps


def _unpack(results):
    eq = np.empty((B, LQ, D), np.float32)
    ea = np.empty((B, LA, D), np.float32)
    for c in range(NCORES):
        out = results[c]
        eq[c * BPC:(c + 1) * BPC] = out["eq_out"].astype(np.float32).reshape(BPC, LQ, D)
        ea[c * BPC:(c + 1) * BPC] = out["ea_out"].astype(np.float32).reshape(BPC, LA, D)
    return eq, ea


# revision 9
# speedup vs baseline: 1.6661x; 1.0743x over previous
"""Cross-encoding kernel for Trainium2 (Bass/Tile), 8-core batch-parallel.

Per batch b:
    query = Q W1 + b1 ; key = A W2 + b2
    S = query key^T / sqrt(d)
    eq = softmax_rows(S) @ A          (qk attention)
    ea = softmax_cols(S)^T @ Q        (kq attention)

Host folds the projections: S_full/sqrt(d) = (Q M) A^T / sqrt(d) + u 1^T
+ 1 v^T with M = W1 W2^T, u = Q W1 b2 / sqrt(d), v = A W2 b1 / sqrt(d)
(the b1.b2 constant cancels in both softmaxes).

On device (per batch, all PE operands bf16):
  ph0: qm = M^T Q^T  (PSUM -> SBUF, resident; no DRAM round trip)
  ph1: for each a-stripe of 256: scores S[q,a] once; ACT exp with
       per-partition bias u -> e; DVE multiply by exp(v) row (replicated
       across partitions) -> E_full tile, with accum_out collecting the
       EQ denominators for free; PE-transpose E_full into the resident
       E^T buffer; EA AV matmuls consume E_full tiles immediately,
       normalized by DVE free-axis reduces of E^T.
  ph2: EQ AV matmuls from E^T x A_nat, normalized by the accumulated
       EQ denominators.

The score matrix is computed ONCE (the old kernel computed both
orientations on the PE); denominators need no ones/fanout matmuls.
"""
import math

import numpy as np

B, LQ, LA, D = 16, 2048, 2048, 1024
NCORES = 8
BPC = B // NCORES

_cached = {}


def _build(lq=LQ, la=LA, d=D, bpc=BPC):
    import concourse.bass as bass
    import concourse.tile as tile
    from concourse import bacc, mybir

    f32 = mybir.dt.float32
    bf16 = mybir.dt.bfloat16
    fp8 = mybir.dt.float8e4
    DR = mybir.MatmulPerfMode.DoubleRow
    ec_n = d // 128          # 8 contraction chunks
    nqt, nat = lq // 128, la // 128   # 16, 16
    nga = la // 256          # 8 a-stripes
    ngq = lq // 256          # 8 q-groups (phase 2)
    nqb = lq // 256          # 8 projection q-blocks of 256
    inv_sqrt_d = 1.0 / math.sqrt(d)

    nc = bacc.Bacc("TRN2", target_bir_lowering=False, debug=False)

    qt_in = nc.dram_tensor("qt_in", [bpc, ec_n, 128, lq], bf16, kind="ExternalInput").ap()
    at_in = nc.dram_tensor("at_in", [bpc, ec_n, 128, la], fp8, kind="ExternalInput").ap()
    qn_in = nc.dram_tensor("qn_in", [bpc, nqt, 128, d], bf16, kind="ExternalInput").ap()
    an_in = nc.dram_tensor("an_in", [bpc, nat, 128, d], bf16, kind="ExternalInput").ap()
    # M dc-major: m_in[et, ec, p, f] = M[ec*128+p, et*128+f]
    m_in = nc.dram_tensor("m_in", [ec_n, ec_n, 128, 128], bf16, kind="ExternalInput").ap()
    id_in = nc.dram_tensor("id_in", [128, 128], bf16, kind="ExternalInput").ap()
    ub_in = nc.dram_tensor("ub_in", [bpc, lq], f32, kind="ExternalInput").ap()
    ev_in = nc.dram_tensor("ev_in", [bpc, 128, la], bf16, kind="ExternalInput").ap()
    eq_out = nc.dram_tensor("eq_out", [bpc, nqt, 128, d], bf16, kind="ExternalOutput").ap()
    ea_out = nc.dram_tensor("ea_out", [bpc, nat, 128, d], bf16, kind="ExternalOutput").ap()

    Exp = mybir.ActivationFunctionType.Exp
    Copy = mybir.ActivationFunctionType.Copy
    AX = mybir.AxisListType.X
    ADD = mybir.AluOpType.add
    MULT = mybir.AluOpType.mult

    with tile.TileContext(nc) as tc:
        with (
            tc.tile_pool(name="const", bufs=1) as constp,
            tc.tile_pool(name="qmp", bufs=1) as qmp,
            tc.tile_pool(name="etp", bufs=1) as etp,
            tc.tile_pool(name="natp", bufs=2) as natp,
            tc.tile_pool(name="qts", bufs=2) as qts,
            tc.tile_pool(name="ats", bufs=2) as ats,
            tc.tile_pool(name="ep", bufs=3) as ep,
            tc.tile_pool(name="stg", bufs=2) as stg,
            tc.tile_pool(name="small", bufs=1) as small,
            tc.tile_pool(name="psO", bufs=2, space=bass.MemorySpace.PSUM) as psO,
            tc.tile_pool(name="psS", bufs=2, space=bass.MemorySpace.PSUM) as psS,
            tc.tile_pool(name="psT", bufs=2, space=bass.MemorySpace.PSUM) as psT,
        ):
            id_sb = constp.tile([128, 128], bf16, tag="id")
            nc.sync.dma_start(out=id_sb, in_=id_in)
            m_sb = constp.tile([128, ec_n, d], bf16, tag="m")
            for et in range(ec_n):
                nc.sync.dma_start(
                    out=m_sb[:, :, et * 128:(et + 1) * 128],
                    in_=m_in[et].rearrange("c p f -> p c f"))

            for bi in range(bpc):
                ub_sb = small.tile([128, nqt], f32, tag="ub")
                nc.sync.dma_start(out=ub_sb, in_=ub_in[bi].rearrange("(t p) -> p t", p=128))
                ev_sb = small.tile([128, la], bf16, tag="ev")
                nc.sync.dma_start(out=ev_sb, in_=ev_in[bi])
                eqd_part = small.tile([128, nqt, nga], f32, tag="eqd")

                # ---- ph0: qm = M^T Q^T, resident in SBUF ----
                qm_sb = qmp.tile([128, ec_n, lq], fp8, tag="qm")
                k = 0
                for qb in range(nqb):
                    qt_t = qts.tile([128, ec_n, 256], bf16, tag="qt", name="qt_t")
                    nc.sync.dma_start(
                        out=qt_t,
                        in_=qt_in[bi][:, :, qb * 256:(qb + 1) * 256]
                        .rearrange("c p q -> p c q"))
                    for et in range(ec_n):
                        pj = psO.tile([128, 1024], f32, tag="acc", name="pj")
                        for ec in range(ec_n):
                            nc.tensor.matmul(
                                pj[:, 0:256],
                                m_sb[:, ec, et * 128:(et + 1) * 128],
                                qt_t[:, ec, :],
                                start=(ec == 0), stop=(ec == ec_n - 1))
                        if k % 2 == 0:
                            nc.vector.tensor_scalar_mul(
                                out=qm_sb[:, et, qb * 256:(qb + 1) * 256],
                                in0=pj[:, 0:256], scalar1=32.0)
                        else:
                            nc.scalar.activation(
                                out=qm_sb[:, et, qb * 256:(qb + 1) * 256],
                                in_=pj[:, 0:256], func=Copy, scale=32.0)
                        k += 1

                # qn for ph1 (EA AV rhs), an for ph2 (EQ AV rhs): same slot
                # system, rotation qn(b) -> an(b) -> qn(b+1) -> ...
                qn_sb = natp.tile([128, nqt, d], bf16, tag="nat", name="qn_sb")
                for blk in range(nqt // 2):
                    nc.sync.dma_start(
                        out=qn_sb[:, blk * 2:(blk + 1) * 2, :],
                        in_=qn_in[bi, blk * 2:(blk + 1) * 2].rearrange("t p d -> p t d"))

                an_sb = natp.tile([128, nat, d], bf16, tag="nat", name="an_sb")
                for blk in range(nat // 2):
                    nc.sync.dma_start(
                        out=an_sb[:, blk * 2:(blk + 1) * 2, :],
                        in_=an_in[bi, blk * 2:(blk + 1) * 2].rearrange("t p d -> p t d"))

                et_sb = etp.tile([128, nat, lq], bf16, tag="et")

                # ---- ph1: stripes over a ----
                for g in range(nga):
                    at_t = ats.tile([128, ec_n, 256], fp8, tag="at", name="at_t")
                    nc.sync.dma_start(
                        out=at_t,
                        in_=at_in[bi][:, :, g * 256:(g + 1) * 256]
                        .rearrange("c p a -> p c a"))
                    pacc = [psO.tile([128, d], f32, tag="acc", name="pacc")
                            for _ in range(2)]

                    def consume(efull, ch, g=g, pacc=pacc):
                        for t2 in range(2):
                            pt = psT.tile([128, 128], bf16, tag="pt", name="pt")
                            nc.tensor.transpose(
                                pt, efull[:, t2 * 128:(t2 + 1) * 128], id_sb)
                            nc.scalar.copy(
                                out=et_sb[:, 2 * g + t2, ch * 128:(ch + 1) * 128],
                                in_=pt)
                            for dh in range(2):
                                nc.tensor.matmul(
                                    pacc[t2][:, dh * 512:(dh + 1) * 512],
                                    efull[:, t2 * 128:(t2 + 1) * 128],
                                    qn_sb[:, ch, dh * 512:(dh + 1) * 512],
                                    start=(ch == 0), stop=(ch == nqt - 1))

                    prev = None
                    for ch in range(nqt):
                        ps = psS.tile([128, 256], f32, tag="s", name="ps")
                        for ej in range(ec_n // 2):
                            nc.tensor.matmul(
                                ps,
                                qm_sb[:, 2 * ej:2 * ej + 2, ch * 128:(ch + 1) * 128],
                                at_t[:, 2 * ej:2 * ej + 2, :],
                                start=(ej == 0), stop=(ej == ec_n // 2 - 1),
                                perf_mode=DR)
                        e_t = ep.tile([128, 256], bf16, tag="e", name="e_t")
                        nc.scalar.activation(
                            out=e_t, in_=ps, func=Exp, scale=inv_sqrt_d / 512.0,
                            bias=ub_sb[:, ch:ch + 1])
                        efull = ep.tile([128, 256], bf16, tag="ef", name="efull")
                        nc.vector.tensor_mul(
                            efull, e_t, ev_sb[:, g * 256:(g + 1) * 256])
                        nc.vector.tensor_reduce(
                            out=eqd_part[:, ch, g:g + 1], in_=efull,
                            axis=AX, op=ADD)
                        if prev is not None:
                            consume(*prev)
                        prev = (efull, ch)
                    consume(*prev)

                    # EA normalize: denominators = free-axis reduce of E^T
                    for t2 in range(2):
                        den = ep.tile([128, 1], f32, tag="dena", name="den")
                        nc.vector.tensor_reduce(
                            out=den, in_=et_sb[:, 2 * g + t2, :], axis=AX, op=ADD)
                        rcp = ep.tile([128, 1], f32, tag="rcpa", name="rcp")
                        nc.vector.reciprocal(out=rcp, in_=den)
                        st = stg.tile([128, d], bf16, tag="st", name="st")
                        nc.scalar.activation(
                            out=st, in_=pacc[t2], func=Copy, scale=rcp)
                        nc.sync.dma_start(out=ea_out[bi, 2 * g + t2], in_=st)

                # EQ denominators: accumulate stripe partials, reciprocal
                eqd = small.tile([128, nqt], f32, tag="eqs")
                nc.vector.tensor_reduce(out=eqd, in_=eqd_part, axis=AX, op=ADD)
                eqr = small.tile([128, nqt], f32, tag="eqr")
                nc.vector.reciprocal(out=eqr, in_=eqd)

                # ---- ph2: EQ AV from E^T ----
                for qg in range(ngq):
                    qacc = [psO.tile([128, d], f32, tag="acc", name="qacc")
                            for _ in range(2)]
                    for ac in range(nat):
                        for t2 in range(2):
                            for dh in range(2):
                                nc.tensor.matmul(
                                    qacc[t2][:, dh * 512:(dh + 1) * 512],
                                    et_sb[:, ac, qg * 256 + t2 * 128: qg * 256 + (t2 + 1) * 128],
                                    an_sb[:, ac, dh * 512:(dh + 1) * 512],
                                    start=(ac == 0), stop=(ac == nat - 1))
                    for t2 in range(2):
                        st = stg.tile([128, d], bf16, tag="st", name="st2")
                        if t2 == 0:
                            nc.vector.tensor_scalar_mul(
                                out=st, in0=qacc[t2],
                                scalar1=eqr[:, 2 * qg + t2: 2 * qg + t2 + 1])
                        else:
                            nc.scalar.activation(
                                out=st, in_=qacc[t2], func=Copy,
                                scale=eqr[:, 2 * qg + t2: 2 * qg + t2 + 1])
                        nc.sync.dma_start(out=eq_out[bi, 2 * qg + t2], in_=st)

    nc.compile()
    return nc


def _get_nc():
    if "nc" not in _cached:
        _cached["nc"] = _build()
    return _cached["nc"]


def _bf16():
    import ml_dtypes
    return ml_dtypes.bfloat16


def _fp8():
    import ml_dtypes
    return ml_dtypes.float8_e4m3fn


def _pack_inputs(Qc, Ac, w1b2, w2b1, lq, la, d):
    bf16 = _bf16()
    ec_n = d // 128
    bpc = Qc.shape[0]
    inv = 1.0 / math.sqrt(d)
    ub = (Qc.astype(np.float64) @ w1b2 * inv).astype(np.float32)     # [bpc, lq]
    v = (Ac.astype(np.float64) @ w2b1 * inv).astype(np.float32)      # [bpc, la]
    ev = np.exp(v).astype(bf16)
    ev_rep = np.broadcast_to(ev[:, None, :], (bpc, 128, la)).copy()
    Qb = Qc.astype(bf16)
    Ab = Ac.astype(bf16)
    return {
        "qt_in": np.ascontiguousarray(Qb.transpose(0, 2, 1)).reshape(bpc, ec_n, 128, lq),
        "at_in": np.ascontiguousarray((Ac * 16.0).astype(_fp8()).transpose(0, 2, 1)).reshape(bpc, ec_n, 128, la),
        "qn_in": np.ascontiguousarray(Qb).reshape(bpc, lq // 128, 128, d),
        "an_in": np.ascontiguousarray(Ab).reshape(bpc, la // 128, 128, d),
        "ub_in": ub,
        "ev_in": ev_rep,
    }


def _fold_weights(W1, b1, W2, b2, d):
    bf16 = _bf16()
    ec_n = d // 128
    M = (W1.astype(np.float64) @ W2.astype(np.float64).T).astype(np.float32)
    w1b2 = W1.astype(np.float64) @ b2.astype(np.float64)
    w2b1 = W2.astype(np.float64) @ b1.astype(np.float64)
    m_packed = np.ascontiguousarray(
        M.astype(bf16).reshape(ec_n, 128, ec_n, 128).transpose(2, 0, 1, 3))
    return m_packed, w1b2, w2b1


def _reference_fallback(Q, A, mask, W1, b1, W2, b2):
    NEG = np.float32(-1e9)
    eqs, eas = [], []
    for b in range(Q.shape[0]):
        query = Q[b] @ W1 + b1
        key = A[b] @ W2 + b2
        s = (query @ key.T) / np.float32(math.sqrt(Q.shape[-1]))
        s = np.where(mask[b] == 0, NEG, s).astype(np.float32)
        sq = s - s.max(axis=1, keepdims=True)
        eq_w = np.exp(sq); eq_w /= eq_w.sum(axis=1, keepdims=True)
        sa = s.T - s.T.max(axis=1, keepdims=True)
        ea_w = np.exp(sa); ea_w /= ea_w.sum(axis=1, keepdims=True)
        eqs.append(eq_w @ A[b])
        eas.append(ea_w @ Q[b])
    return np.stack(eqs), np.stack(eas)


def kernel(Q, A, mask, W1, b1, W2, b2):
    Q = np.ascontiguousarray(Q, dtype=np.float32)
    A = np.ascontiguousarray(A, dtype=np.float32)
    W1 = np.ascontiguousarray(W1, dtype=np.float32)
    W2 = np.ascontiguousarray(W2, dtype=np.float32)
    b1 = np.ascontiguousarray(b1, dtype=np.float32)
    b2 = np.ascontiguousarray(b2, dtype=np.float32)

    if not np.all(mask == 1):
        return _reference_fallback(Q, A, mask, W1, b1, W2, b2)

    from concourse import bass_utils

    nc = _get_nc()
    in_maps = _make_in_maps(Q, A, W1, b1, W2, b2)
    res = bass_utils.run_bass_kernel_spmd(nc, in_maps, core_ids=list(range(NCORES)))
    return _unpack(res.results)


def _make_in_maps(Q, A, W1, b1, W2, b2):
    bf16 = _bf16()
    m_packed, w1b2, w2b1 = _fold_weights(W1, b1, W2, b2, D)
    id128 = np.eye(128, dtype=bf16)
    in_maps = []
    for c in range(NCORES):
        sl = slice(c * BPC, (c + 1) * BPC)
        m = _pack_inputs(Q[sl], A[sl], w1b2, w2b1, LQ, LA, D)
        m.update({"m_in": m_packed, "id_in": id128})
        in_maps.append(m)
    return in_maps


def _unpack(results):
    eq = np.empty((B, LQ, D), np.float32)
    ea = np.empty((B, LA, D), np.float32)
    for c in range(NCORES):
        out = results[c]
        eq[c * BPC:(c + 1) * BPC] = out["eq_out"].astype(np.float32).reshape(BPC, LQ, D)
        ea[c * BPC:(c + 1) * BPC] = out["ea_out"].astype(np.float32).reshape(BPC, LA, D)
    return eq, ea
